# revision 46
# baseline (speedup 1.0000x reference)
"""Adaptive wavelet transform (db8 DWT -> quantile threshold mask -> IDWT) on
Trainium2, 8 NeuronCores, batch-sharded (4 batches per core).

Self-contained: hardcoded shapes [32, 4096, 512]; db8 filter taps inlined.

The mask decision (cd^2 > quantile-threshold) must match the XLA-CPU reference
bit-exactly: any flipped coefficient at the threshold boundary costs ~5e-2
relative error, over the 2e-2 gate.  XLA CPU computes the DWT conv as a
sequential fused-FMA chain over the 16 taps (acc = fmaf(k[j], x[1+2i+j], acc)),
which the PE cannot reproduce (fp32 matmuls go through limb decomposition,
~1e-6 error).  So:

  P1 (per batch): PE computes ca (lowpass) via banded-matrix matmuls (values
      only feed the reconstruction; ~2e-6 error is fine) and spills to DRAM.
      PE also transposes x into channel-major; the DVE then computes cd
      BIT-EXACTLY with a Dekker-style emulation of the fused-FMA chain:
      per tap, p = RN(h*x) and its exact error e (via Veltkamp-split x and h),
      TwoSum with the accumulator, y = s + (t + e).  All ops are single-rounded
      fp32 on the DVE datapath; validated 100% bit-exact vs jax CPU.
  P2 (per batch): exact per-row quantile via bit-space ternary search on the
      squared cd (monotone int32 view of nonneg floats), fused 2-probe count
      per pass (custom DVE op); then v_k / conditional-min v_{k+1} and a
      Dekker two-product/two-sum replicating XLA CPU's fused
      thr = fma(v_k, 1-g, RN(v_{k+1}*g)) bit-exactly.  The mask is applied in
      channel-major (per-partition threshold), PE-transposed to [W, C], and
      spilled.
  P3 (per batch): per 64-sample output chunk, load ca + masked-cd rows from
      DRAM, one IDWT matmul, DMA PSUM->DRAM out.
"""
import numpy as np

B, T, C = 32, 4096, 512
FLEN = 16
W = 2055
TE = T + 2 * (FLEN - 1)   # 4126 extended (symmetric-pad) cols; conv reads 1..4124
NB = 4            # batches per core
NCORES = 8
NCT = C // 128    # 4 channel tiles per batch

DB8_DEC_LO = np.array([
    -0.00011747678400228192, 0.0006754494059985568, -0.0003917403729959771,
    -0.00487035299301066, 0.008746094047015655, 0.013981027917015516,
    -0.04408825393106472, -0.01736930100202211, 0.128747426620186,
    0.00047248457399797254, -0.2840155429624281, -0.015829105256023893,
    0.5853546836548691, 0.6756307362980128, 0.3128715909144659,
    0.05441584224308161], dtype=np.float32)
_signs = ((-1.0) ** np.arange(FLEN)).astype(np.float32)
KHI = (DB8_DEC_LO * _signs).astype(np.float32)   # conv taps for cd, j ascending
DEC_LO = DB8_DEC_LO.astype(np.float64)
REC_LO = DEC_LO[::-1].copy()
REC_HI = (DEC_LO * _signs.astype(np.float64))
DEC_HI = REC_HI[::-1].copy()


# ------------------------- host-side constant builders ----------------------

def _mirror(t):
    t = np.asarray(t)
    t = np.where(t < 0, -1 - t, t)
    t = np.where(t >= T, 2 * T - 1 - t, t)
    return t


def build_dwt_matrix(h):
    A = np.zeros((T, W), dtype=np.float64)
    for i in range(W):
        for j in range(FLEN):
            A[_mirror(1 + 2 * i - j), i] += h[j]
    return A


def build_idwt_matrix(rec):
    R = np.zeros((W, T), dtype=np.float64)
    for i in range(W):
        for t in range(max(0, 2 * i - 14), min(T, 2 * i + 2)):
            j = t + 14 - 2 * i
            if 0 <= j < FLEN:
                R[i, t] += rec[j]
    return R


_HC = None


def host_consts():
    global _HC
    if _HC is not None:
        return _HC
    A_lo = build_dwt_matrix(DEC_LO)
    A_hi = build_dwt_matrix(DEC_HI)
    R_lo = build_idwt_matrix(REC_LO)
    R_hi = build_idwt_matrix(REC_HI)
    c = {}

    def ab(o):
        cols = slice(64 * o, 64 * o + 64)
        return np.concatenate([A_lo[:, cols], A_hi[:, cols]], axis=1)  # [T,128]

    c["MB0"] = ab(0)[0:128].astype(np.float32)
    c["MBi"] = ab(1)[128:256].astype(np.float32)
    c["MAi"] = ab(1)[0:128].astype(np.float32)
    assert np.abs(c["MAi"][64:114]).max() == 0
    tl = np.concatenate([A_lo[:, 2048:W], np.zeros((T, 57)),
                         A_hi[:, 2048:W]], axis=1)  # [T, 71]
    assert np.abs(tl[:3968]).max() == 0
    c["MT"] = tl[3968:4096].astype(np.float32)

    supports = []
    for v in range(64):
        cols = slice(64 * v, 64 * v + 64)
        nz = np.nonzero(np.abs(R_lo[:, cols]).sum(1) + np.abs(R_hi[:, cols]).sum(1))[0]
        supports.append((int(nz[0]), int(nz[-1] + 1)))
    for v in range(64):
        assert supports[v] == (32 * v, min(32 * v + 39, W)), (v, supports[v])
    c["IDWT_SUPPORT"] = supports

    def rblk(v):
        i0, i1 = supports[v]
        cols = slice(64 * v, 64 * v + 64)
        blk = np.concatenate([R_lo[i0:i1, cols], np.zeros((25, 64)),
                              R_hi[i0:i1, cols]], axis=0)
        return np.ascontiguousarray(blk.astype(np.float32))  # [103, 64]

    c["RBi"] = rblk(1)
    for v in (0, 2, 33, 62, 63):
        assert np.array_equal(rblk(v), c["RBi"])
    # split lo/hi blocks for the pair-chunk IDWT: [64,64] at base 0 and the
    # same content at partition base 32 (matmul needs lhsT/rhs bases equal)
    rl64 = np.ascontiguousarray(c["RBi"][0:64])            # 39 lo rows + 25 zero
    rh64 = np.concatenate([c["RBi"][64:103], np.zeros((25, 64), np.float32)])
    c["RL_A"] = rl64
    c["RH_A"] = np.ascontiguousarray(rh64)
    c["RL_B"] = np.concatenate([np.zeros((32, 64), np.float32), rl64])
    c["RH_B"] = np.concatenate([np.zeros((32, 64), np.float32), rh64])
    c["IDENT"] = np.eye(128, dtype=np.float32)
    _HC = c
    return c


CONST_NAMES = ("MB0", "MBi", "MAi", "MT", "RL_A", "RH_A",
               "RL_B", "RH_B", "IDENT")


def quantile_host_params(q):
    q = np.float32(q)
    n = np.float32(W)
    pos = np.float32(q * (n - np.float32(1.0)))
    low = np.float32(np.floor(pos))
    g = np.float32(pos - low)
    lw = np.float32(np.float32(1.0) - g)
    return int(low), float(g), float(lw)


def bisect_schedule():
    d = np.float32(512.0).view(np.int32).item() + 1
    ts = []
    while d > 1:
        t = (d + 2) // 3
        ts.append(t)
        d = t
    return ts


def veltkamp_split(b):
    b = np.float32(b)
    t = np.float32(b * np.float32(4097.0))
    bhi = np.float32(t - np.float32(t - b))
    blo = np.float32(b - bhi)
    return float(bhi), float(blo)


# ----------------------------- custom DVE ops -------------------------------

_OPS = {}


def _register_ops():
    if _OPS:
        return _OPS
    import concourse.dve_ops as D
    from concourse.dve_spec import (Spec, Src0, Src1, C0, C1, C2, Zero, sq,
                                    select, lower, minn)
    from concourse.dve_spec import _has_src1 as has_src1
    from concourse.dve_uop import DveOpSpec
    from operator import add as _add

    def reg(name, spec, subdim=False):
        if name in D._SUB_OPCODE_FOR_NAME:
            return next(o for o in D.OPS if o.name == name)
        row = max(D._SUB_OPCODE_FOR_NAME.values()) + 1
        assert row < 0x20
        D._SUB_OPCODE_FOR_NAME[name] = row
        shas = {}
        for ver in ("v3", "v4"):
            r = DveOpSpec(name=name, opcode=row, uops=lower(spec, ver=ver),
                          rd1_en=has_src1(spec))
            shas[ver] = r.sha(ver)
        op = D.DveOp(name, spec, subdim, uops_sha=shas)
        D.OPS.append(op)
        D.CUSTOM_DVE_SPECS[name] = spec
        return op

    f32 = np.float32

    def _count2_ref(in0, in1, s0, s1, imm2):
        x = in0.astype(np.float32) ** 2
        body = ((x <= s0).astype(np.float32) + (x <= s1) * np.float32(imm2))
        return body, body.reshape(body.shape[0], -1).sum(-1, keepdims=True)

    def _condmin_ref(in0, in1, s0, s1, imm2):
        x = in0.astype(np.float32) ** 2
        body = np.where(x > s0, x, np.float32(imm2)).astype(np.float32)
        return body, body.reshape(body.shape[0], -1).min(-1, keepdims=True)

    def _e1_ref(in0, in1, s0, s1, imm2):
        # in0=x, in1=xl, s0=h, s1=hh, imm2=hl; xh = x - xl (exact)
        xh = (in0 - in1).astype(f32)
        p = (f32(s0) * in0).astype(f32)
        t1 = (f32(s1) * xh).astype(f32)
        t2 = (t1 - p).astype(f32)
        t3 = (f32(s1) * in1).astype(f32)
        t4 = (t2 + t3).astype(f32)
        t5 = (f32(imm2) * xh).astype(f32)
        return (t4 + t5).astype(f32)

    def _axpy_ref(in0, in1, s0, s1, imm2):
        return (in0 + (f32(s0) * in1).astype(f32)).astype(f32)

    def _twosumt_ref(in0, in1, s0, s1, imm2):
        s = (in0 + in1).astype(f32)
        bb = (s - in0).astype(f32)
        u = (s - bb).astype(f32)
        v = (in0 - u).astype(f32)
        w = (in1 - bb).astype(f32)
        return (v + w).astype(f32)

    def _splitlo_ref(in0, in1, s0, s1, imm2):
        t = (in0 * f32(s0)).astype(f32)
        d = (t - in0).astype(f32)
        xh = (t - d).astype(f32)
        return (in0 - xh).astype(f32)

    def _cdmask_ref(in0, in1, s0, s1, imm2):
        x = in0.astype(np.float32) ** 2
        return np.where(x > s0, in0, np.float32(0.0)).astype(f32)

    _s = sq(Src0)
    _OPS["COUNT2"] = reg(
        "AWT_COUNT2",
        Spec(body=(_s <= C0) + (_s <= C1) * C2, accum=_add, accum_init=Zero,
             reference=_count2_ref))
    _OPS["CONDMIN"] = reg(
        "AWT_CONDMIN",
        Spec(body=select(sq(Src0) > C0, sq(Src0), C2), accum=minn,
             accum_init=C1, reference=_condmin_ref))
    # E1: Dekker product-error head with in0=x, in1=xl (xh derived, exact):
    #     ((hh*xh - h*x) + hh*xl) + hl*xh
    _xh = Src0 - Src1
    _p = C0 * Src0
    _t2 = C1 * _xh - _p
    _t4 = _t2 + C1 * Src1
    _e1 = _t4 + C2 * _xh
    _OPS["E1"] = reg("AWT_E1", Spec(body=_e1, reference=_e1_ref))
    _OPS["AXPY"] = reg("AWT_AXPY", Spec(body=Src0 + C0 * Src1,
                                        reference=_axpy_ref))
    _ss = Src0 + Src1
    _bb = _ss - Src0
    _tt = (Src0 - (_ss - _bb)) + (Src1 - _bb)
    _OPS["TWOSUMT"] = reg("AWT_TWOSUMT", Spec(body=_tt, reference=_twosumt_ref))
    _t_ = Src0 * C0
    _OPS["SPLITLO"] = reg(
        "AWT_SPLITLO",
        Spec(body=Src0 - (_t_ - (_t_ - Src0)), reference=_splitlo_ref))
    _OPS["CDMASK"] = reg(
        "AWT_CDMASK",
        Spec(body=select(sq(Src0) > C0, Src0, Zero), reference=_cdmask_ref))

    def _probl_ref(in0, in1, s0, s1, imm2):
        a = (in0 + f32(s0)).astype(f32)
        c = (a >= f32(s1)).astype(f32)
        return (a - (c * f32(s1)).astype(f32)).astype(f32)

    def _probh_ref(in0, in1, s0, s1, imm2):
        a = (in0 + f32(s0)).astype(f32)
        c = (a >= f32(s1)).astype(f32)
        b = (in1 + f32(imm2)).astype(f32)
        return (b + c).astype(f32)

    _a = Src0 + C0
    _OPS["PROBL"] = reg("AWT_PROBL",
                        Spec(body=_a - (_a >= C1) * C1, reference=_probl_ref))
    _a2 = Src0 + C0
    _OPS["PROBH"] = reg("AWT_PROBH",
                        Spec(body=(Src1 + C2) + (_a2 >= C1),
                             reference=_probh_ref))
    return _OPS


# ----------------------------- device kernel --------------------------------

def build_nc(k, g, lw, debug=False):
    import concourse.bass as bass  # noqa: F401
    import concourse.tile as tile
    from concourse import bacc, mybir
    from contextlib import ExitStack

    ops = _register_ops()
    COUNT2, CONDMIN = ops["COUNT2"], ops["CONDMIN"]
    E1, AXPY, TWOSUMT = ops["E1"], ops["AXPY"], ops["TWOSUMT"]
    SPLITLO, CDMASK = ops["SPLITLO"], ops["CDMASK"]
    PROBL, PROBH = ops["PROBL"], ops["PROBH"]
    hc = host_consts()
    TS = bisect_schedule()
    kp1 = float(k + 1)
    kp2 = float(k + 2)
    f32 = mybir.dt.float32
    i32 = mybir.dt.int32
    AL = mybir.AluOpType
    supports = hc["IDWT_SUPPORT"]
    bhi_, blo_ = veltkamp_split(lw)
    tap_splits = [veltkamp_split(KHI[j]) for j in range(FLEN)]

    nc = bacc.Bacc("TRN2", debug=False, enable_asserts=False)
    x_ap = nc.dram_tensor("x", [NB, T, C], f32, kind="ExternalInput").ap()
    out_ap = nc.dram_tensor("out", [NB, T, C], f32, kind="ExternalOutput").ap()
    spill_kind = "ExternalOutput" if debug else "Internal"
    WPAD = 2080
    cad2 = [nc.dram_tensor(f"cacd_d{b}", [WPAD, 2 * C], f32,
                           kind=spill_kind).ap() for b in range(NB)]
    dbg_thr = (nc.dram_tensor("dbg_thr", [128, NB * NCT], f32,
                              kind="ExternalOutput").ap() if debug else None)
    consts = {n: nc.dram_tensor(n.lower(), list(hc[n].shape), f32,
                                kind="ExternalInput").ap() for n in CONST_NAMES}

    with tile.TileContext(nc) as tc, ExitStack() as ctx:
        cpool = ctx.enter_context(tc.tile_pool(name="consts", bufs=1))
        xpool = ctx.enter_context(tc.tile_pool(name="x", bufs=2))
        xcpool = ctx.enter_context(tc.tile_pool(name="xc", bufs=2))
        xepool = ctx.enter_context(tc.tile_pool(name="xe", bufs=1))
        wpool = ctx.enter_context(tc.tile_pool(name="wrk", bufs=1))
        tpool = ctx.enter_context(tc.tile_pool(name="tmp", bufs=2))
        cdtp = ctx.enter_context(tc.tile_pool(name="cdt", bufs=1))
        capool = ctx.enter_context(tc.tile_pool(name="cap", bufs=2))
        stp = ctx.enter_context(tc.tile_pool(name="state", bufs=1))
        tsp = ctx.enter_context(tc.tile_pool(name="tstage", bufs=1))
        vp = ctx.enter_context(tc.tile_pool(name="vt", bufs=1))
        dwtps = ctx.enter_context(tc.tile_pool(name="dwtps", bufs=2, space="PSUM"))
        trps = ctx.enter_context(tc.tile_pool(name="trps", bufs=2, space="PSUM"))
        idps = ctx.enter_context(tc.tile_pool(name="idps", bufs=2, space="PSUM"))

        ct = {}
        for name in CONST_NAMES:
            t_ = cpool.tile(list(hc[name].shape), f32, tag=name)
            nc.sync.dma_start(t_[:], consts[name][:])
            ct[name] = t_

        def st(tag, dtype=f32, cols=NCT):
            return stp.tile([128, cols], dtype, tag=tag, name=tag)

        cdt = [cdtp.tile([128, W], f32, tag=f"cd{cb}", name=f"cd{cb}")
               for cb in range(NCT)]
        zt = cpool.tile([32, C], f32, tag="ZPAD", name="ZPAD")
        nc.vector.memset(zt[:], 0.0)
        xl_ts = [wpool.tile([128, TE], f32, tag=f"xl{i}", name=f"xl{i}")
                 for i in range(2)]
        yts = [wpool.tile([128, W], f32, tag=f"yt{i}", name=f"yt{i}")
               for i in range(2)]

        # ------------- P1a: PE ca-matmul + spill, per batch ------------------
        def emit_p1_ca(b):
            xt = {}
            capair = None
            for o in range(33):
                if o < 32:
                    xt[o] = xpool.tile([128, C], f32, tag="x", name="x")
                    nc.sync.dma_start(xt[o][:],
                                      x_ap[b, 128 * o:128 * o + 128, :])
                    ps = dwtps.tile([128, C], f32, tag="dwt", name="dwt")
                    mb = ct["MB0"] if o == 0 else ct["MBi"]
                    nc.tensor.matmul(ps[:], mb[:], xt[o][:],
                                     start=True, stop=(o == 0))
                    if o > 0:
                        nc.tensor.matmul(ps[:], ct["MAi"][64:128, :],
                                         xt[o - 1][64:128, :],
                                         start=False, stop=True)
                        xt.pop(o - 1)
                    nrow = 64
                else:
                    ps = dwtps.tile([128, C], f32, tag="dwt", name="dwt")
                    nc.tensor.matmul(ps[0:71, :], ct["MT"][:], xt[31][:],
                                     start=True, stop=True)
                    nrow = 7
                if o % 2 == 0:
                    capair = capool.tile([128, C], f32, tag="cat", name="cat")
                r = 64 * (o % 2)
                nc.scalar.copy(capair[r:r + nrow, :], ps[0:nrow, :])
                if o % 2 == 1 or o == 32:
                    w_ = o // 2
                    nrows = 128 if w_ < 16 else W - 2048
                    nc.scalar.dma_start(
                        cad2[b][128 * w_:128 * w_ + nrows, 0:C],
                        capair[0:nrows, :])
            # zero the pad rows (W..WPAD) of both halves once per batch
            nc.sync.dma_start(cad2[b][W:WPAD, 0:C], zt[0:WPAD - W, :])
            nc.scalar.dma_start(cad2[b][W:WPAD, C:2 * C], zt[0:WPAD - W, :])

        # ------------- P1b: build xT_ext for a (b, cb-pair) ------------------
        def emit_build_xe_pair(b, cbs):
            xes = [xepool.tile([128, TE], f32, tag=f"xe{cb % 2}",
                               name=f"xe{cb % 2}") for cb in cbs]
            for o in range(32):
                xc = xcpool.tile([128, 256], f32, tag="xc", name="xc")
                nc.sync.dma_start(
                    xc[:], x_ap[b, 128 * o:128 * o + 128,
                                128 * cbs[0]:128 * cbs[0] + 256])
                for i in range(2):
                    tp = trps.tile([128, 128], f32, tag="tr", name="tr")
                    nc.tensor.transpose(tp[:], xc[:, 128 * i:128 * i + 128],
                                        ct["IDENT"][:])
                    nc.scalar.copy(xes[i][:, 15 + 128 * o:15 + 128 * o + 128],
                                   tp[:])
            # mirror columns: head m=0..14 <- col 29-m ; tail 4111+i <- 4110-i
            for xe in xes:
                for m in range(15):
                    nc.scalar.copy(xe[:, m:m + 1], xe[:, 29 - m:30 - m])
                    nc.scalar.copy(xe[:, 4111 + m:4112 + m],
                                   xe[:, 4110 - m:4111 - m])
            return xes

        # ------------- P1c: exact DVE conv for one (b, cb) -------------------
        def emit_conv_pair(b, cbs, xes):
            # Two channel-tiles interleaved so engine busy-time dominates the
            # per-tap cross-engine latency chain.  Engine split (all fp32 ops
            # verified bit-exact RN on device):
            #   ACT:  p = h*x          Pool: s = p+y, r = t+e
            #   DVE:  SPLITLO, E1, AXPY, TWOSUMT (custom), y = s+r
            for i, (cb, xe) in enumerate(zip(cbs, xes)):
                nc.vector._custom_dve(SPLITLO, out=xl_ts[i][:], in0=xe[:],
                                      s0=4097.0)

            def sl(tile_, j):
                return tile_[:, 1 + j:1 + j + 2 * W:2]

            for i, (cb, xe) in enumerate(zip(cbs, xes)):
                nc.scalar.mul(yts[i][:], sl(xe, 0), float(KHI[0]))
            for j in range(1, FLEN):
                h = float(KHI[j])
                hh, hl = tap_splits[j]
                for i, (cb, xe) in enumerate(zip(cbs, xes)):
                    xl = xl_ts[i]
                    y = yts[i]
                    ynext = cdt[cb] if j == FLEN - 1 else y
                    p_ = tpool.tile([128, W], f32, tag="p", name="p")
                    e_ = tpool.tile([128, W], f32, tag="e", name="e")
                    s_ = tpool.tile([128, W], f32, tag="s", name="s")
                    t_ = tpool.tile([128, W], f32, tag="t", name="t")
                    nc.scalar.mul(p_[:], sl(xe, j), h)
                    nc.vector._custom_dve(E1, out=e_[:], in0=sl(xe, j),
                                          in1=sl(xl, j), s0=h, s1=hh, imm2=hl)
                    nc.vector._custom_dve(AXPY, out=e_[:], in0=e_[:],
                                          in1=sl(xl, j), s0=hl)
                    nc.gpsimd.tensor_tensor(out=s_[:], in0=p_[:], in1=y[:],
                                            op=AL.add)
                    nc.vector._custom_dve(TWOSUMT, out=t_[:], in0=p_[:],
                                          in1=y[:])
                    nc.gpsimd.tensor_tensor(out=t_[:], in0=t_[:], in1=e_[:],
                                            op=AL.add)
                    nc.vector.tensor_tensor(out=ynext[:], in0=s_[:],
                                            in1=t_[:], op=AL.add)

        # ------------- P2: per-batch bisection + threshold -------------------
        def emit_bisect(b):
            lol, loh = st(f"lol{b}"), st(f"loh{b}")
            nc.vector.memset(lol[:], 0.0)
            nc.vector.memset(loh[:], 0.0)
            comb = st("comb")
            c1t, e1c, e2c, sf = st("c1t"), st("e1c"), st("e2c"), st("sf")
            tmp, tmp2, carry = st("tmp"), st("tmp2"), st("carry")
            ci = st("ci", i32)
            k4095 = st("k4095", i32)
            nc.vector.memset(k4095[:], 4095)
            k65535 = st("k65535", i32)
            nc.vector.memset(k65535[:], 65535)
            m1l, m1h, m2l, m2h = st("m1l"), st("m1h"), st("m2l"), st("m2h")
            m1i, m2i = st("m1i", i32), st("m2i", i32)
            ih = st("ih", i32)

            def mk_probe(ml, mh, mi, off):
                offl, offh = float(off & 0xFFFF), float(off >> 16)
                nc.vector._custom_dve(PROBL, out=ml[:], in0=lol[:],
                                      s0=offl, s1=65536.0)
                nc.vector._custom_dve(PROBH, out=mh[:], in0=lol[:],
                                      in1=loh[:], s0=offl, s1=65536.0,
                                      imm2=offh)
                nc.vector.tensor_copy(ih[:], mh[:])
                nc.vector.tensor_scalar(out=ih[:], in0=ih[:], scalar1=16,
                                        scalar2=None,
                                        op0=AL.logical_shift_left)
                nc.vector.tensor_copy(mi[:], ml[:])
                nc.vector.tensor_tensor(out=mi[:], in0=mi[:], in1=ih[:],
                                        op=AL.bitwise_or)

            for Tstep in TS:
                mk_probe(m1l, m1h, m1i, Tstep - 1)
                mk_probe(m2l, m2h, m2i, 2 * Tstep - 1)
                m1f = m1i[:].bitcast(f32)
                m2f = m2i[:].bitcast(f32)
                for cb in range(NCT):
                    sc = tpool.tile([128, W], f32, tag="p", name="cscr")
                    nc.vector._custom_dve(
                        COUNT2, out=sc[:], accum_out=comb[:, cb:cb + 1],
                        in0=cdt[cb][:], s0=m1f[:, cb:cb + 1],
                        s1=m2f[:, cb:cb + 1], imm2=4096.0)
                nc.vector.tensor_copy(ci[:], comb[:])
                nc.vector.tensor_tensor(out=ci[:], in0=ci[:], in1=k4095[:],
                                        op=AL.bitwise_and)
                nc.vector.tensor_copy(c1t[:], ci[:])
                nc.vector.tensor_scalar(out=e1c[:], in0=c1t[:], scalar1=kp1,
                                        scalar2=None, op0=AL.is_lt)
                nc.vector.tensor_scalar(out=e2c[:], in0=comb[:],
                                        scalar1=4096.0 * kp1, scalar2=None,
                                        op0=AL.is_lt)
                nc.vector.tensor_tensor(out=sf[:], in0=e1c[:], in1=e2c[:],
                                        op=AL.add)
                tl_, th_ = float(Tstep & 0xFFFF), float(Tstep >> 16)
                nc.vector.tensor_scalar(out=tmp[:], in0=sf[:], scalar1=tl_,
                                        scalar2=None, op0=AL.mult)
                nc.vector.tensor_tensor(out=lol[:], in0=lol[:], in1=tmp[:],
                                        op=AL.add)
                nc.vector.tensor_copy(ci[:], lol[:])
                nc.vector.tensor_tensor(out=ci[:], in0=ci[:], in1=k65535[:],
                                        op=AL.bitwise_and)
                nc.vector.tensor_copy(tmp2[:], ci[:])
                nc.vector.tensor_tensor(out=carry[:], in0=lol[:], in1=tmp2[:],
                                        op=AL.subtract)
                nc.vector.tensor_scalar(out=carry[:], in0=carry[:],
                                        scalar1=1.0 / 65536.0, scalar2=None,
                                        op0=AL.mult)
                nc.vector.tensor_copy(lol[:], tmp2[:])
                if th_ != 0.0:
                    nc.vector.tensor_scalar(out=tmp[:], in0=sf[:], scalar1=th_,
                                            scalar2=None, op0=AL.mult)
                    nc.vector.tensor_tensor(out=loh[:], in0=loh[:],
                                            in1=tmp[:], op=AL.add)
                nc.vector.tensor_tensor(out=loh[:], in0=loh[:], in1=carry[:],
                                        op=AL.add)

            vk = st("vk", i32)
            nc.vector.tensor_copy(ih[:], loh[:])
            nc.vector.tensor_scalar(out=ih[:], in0=ih[:], scalar1=16,
                                    scalar2=None, op0=AL.logical_shift_left)
            nc.vector.tensor_copy(vk[:], lol[:])
            nc.vector.tensor_tensor(out=vk[:], in0=vk[:], in1=ih[:],
                                    op=AL.bitwise_or)
            vkf = vk[:].bitcast(f32)

            cvk = st("cvk")
            vnext = st("vnext")
            for cb in range(NCT):
                sc = tpool.tile([128, W], f32, tag="p", name="cscr")
                nc.vector._custom_dve(
                    COUNT2, out=sc[:], accum_out=cvk[:, cb:cb + 1],
                    in0=cdt[cb][:], s0=vkf[:, cb:cb + 1], s1=512.0,
                    imm2=4096.0)
                sc2 = tpool.tile([128, W], f32, tag="e", name="cscr2")
                nc.vector._custom_dve(
                    CONDMIN, out=sc2[:], accum_out=vnext[:, cb:cb + 1],
                    in0=cdt[cb][:], s0=vkf[:, cb:cb + 1], s1=3.0e38,
                    imm2=3.0e38)
            nc.vector.tensor_scalar(out=cvk[:], in0=cvk[:],
                                    scalar1=-float(W) * 4096.0, scalar2=None,
                                    op0=AL.add)
            vh = st("vh")
            if g == 0.0:
                nc.vector.tensor_copy(vh[:], vkf)
            else:
                msk = st("msk")
                mski = st("mski", i32)
                nc.vector.tensor_scalar(out=msk[:], in0=cvk[:], scalar1=kp2,
                                        scalar2=None, op0=AL.is_ge)
                nc.vector.tensor_copy(mski[:], msk[:])
                nc.vector.tensor_copy(vh[:], vnext[:])
                nc.vector.copy_predicated(vh[:], mski[:], vkf)

            # thr = fma(vk, lw, RN(vh*g)) via Dekker two-product + two-sum
            thr = st(f"thr{b}")
            cc, tt2, ah, al2 = st("cc"), st("tt2"), st("ah"), st("al2")
            ph, er, ss_, bb, t1 = st("ph"), st("er"), st("ss_"), st("bb"), st("t1")
            nc.vector.tensor_scalar(out=cc[:], in0=vh[:], scalar1=float(g),
                                    scalar2=None, op0=AL.mult)
            nc.vector.tensor_scalar(out=tt2[:], in0=vkf, scalar1=4097.0,
                                    scalar2=None, op0=AL.mult)
            nc.vector.tensor_tensor(out=ah[:], in0=tt2[:], in1=vkf,
                                    op=AL.subtract)
            nc.vector.tensor_tensor(out=ah[:], in0=tt2[:], in1=ah[:],
                                    op=AL.subtract)
            nc.vector.tensor_tensor(out=al2[:], in0=vkf, in1=ah[:],
                                    op=AL.subtract)
            nc.vector.tensor_scalar(out=ph[:], in0=vkf, scalar1=float(lw),
                                    scalar2=None, op0=AL.mult)
            nc.vector.tensor_scalar(out=er[:], in0=ah[:], scalar1=bhi_,
                                    scalar2=None, op0=AL.mult)
            nc.vector.tensor_tensor(out=er[:], in0=er[:], in1=ph[:],
                                    op=AL.subtract)
            for a_, b_ in ((ah, blo_), (al2, bhi_), (al2, blo_)):
                nc.vector.tensor_scalar(out=tmp[:], in0=a_[:], scalar1=b_,
                                        scalar2=None, op0=AL.mult)
                nc.vector.tensor_tensor(out=er[:], in0=er[:], in1=tmp[:],
                                        op=AL.add)
            nc.vector.tensor_tensor(out=ss_[:], in0=ph[:], in1=cc[:], op=AL.add)
            nc.vector.tensor_tensor(out=bb[:], in0=ss_[:], in1=ph[:],
                                    op=AL.subtract)
            nc.vector.tensor_tensor(out=t1[:], in0=cc[:], in1=bb[:],
                                    op=AL.subtract)
            nc.vector.tensor_tensor(out=bb[:], in0=ss_[:], in1=bb[:],
                                    op=AL.subtract)
            nc.vector.tensor_tensor(out=bb[:], in0=ph[:], in1=bb[:],
                                    op=AL.subtract)
            nc.vector.tensor_tensor(out=t1[:], in0=t1[:], in1=bb[:], op=AL.add)
            nc.vector.tensor_tensor(out=t1[:], in0=t1[:], in1=er[:], op=AL.add)
            nc.vector.tensor_tensor(out=thr[:], in0=ss_[:], in1=t1[:],
                                    op=AL.add)
            if debug:
                nc.sync.dma_start(dbg_thr[:, NCT * b:NCT * (b + 1)], thr[:])
            return thr

        # ------------- P2b: mask in channel-major, transpose, spill ----------
        def emit_mask_spill(b, thr):
            for cb in range(NCT):
                nc.vector._custom_dve(CDMASK, out=cdt[cb][:], in0=cdt[cb][:],
                                      s0=thr[:, cb:cb + 1])
            for wb in range(17):
                cc = min(128, W - 128 * wb)
                stg = tsp.tile([128, C], f32, tag="ts", name="ts")
                for cb in range(NCT):
                    tp = trps.tile([128, 128], f32, tag="tr", name="tr")
                    nc.tensor.transpose(tp[0:cc, :],
                                        cdt[cb][:, 128 * wb:128 * wb + cc],
                                        ct["IDENT"][:])
                    nc.scalar.copy(stg[0:cc, 128 * cb:128 * cb + 128],
                                   tp[0:cc, :])
                nc.scalar.dma_start(
                    cad2[b][128 * wb:128 * wb + cc, C:2 * C], stg[0:cc, :])

        # ------------- P3: IDWT, per batch (2 output chunks per iter) --------
        def emit_p3(b):
            for vp_ in range(32):
                r0 = 64 * vp_
                vta = vp.tile([64, 2 * C], f32, tag="vt", name="vt")
                nc.sync.dma_start(vta[:], cad2[b][r0:r0 + 64, :])
                vtb = vp.tile([64, 2 * C], f32, tag="vt2", name="vt2")
                nc.sync.dma_start(vtb[:], cad2[b][r0 + 32:r0 + 96, :])
                psa = idps.tile([64, C], f32, tag="idwta", name="idwta")
                nc.tensor.matmul(psa[:], ct["RL_A"][:], vta[:, 0:C],
                                 start=True, stop=False)
                nc.tensor.matmul(psa[:], ct["RH_A"][:], vta[:, C:2 * C],
                                 start=False, stop=True)
                psb = idps.tile([64, C], f32, tag="idwtb", name="idwtb")
                nc.tensor.matmul(psb[:], ct["RL_A"][:], vtb[:, 0:C],
                                 start=True, stop=False)
                nc.tensor.matmul(psb[:], ct["RH_A"][:], vtb[:, C:2 * C],
                                 start=False, stop=True)
                ot = vp.tile([128, C], f32, tag="ot", name="ot")
                nc.scalar.copy(ot[0:64, :], psa[:])
                nc.scalar.copy(ot[64:128, :], psb[:])
                nc.scalar.dma_start(out_ap[b, 128 * vp_:128 * vp_ + 128, :],
                                    ot[:])

        # ----------------------------- schedule ------------------------------
        # Software pipeline: the next batch's P1/xe-build (DMA/PE/ACT only,
        # no DVE ops) is emitted right after conv(b, pair2) so it overlaps
        # bisect/mask/P3 of batch b instead of queueing behind them.
        xes_next = emit_build_xe_pair(0, (0, 1))
        emit_p1_ca(0)
        for b in range(NB):
            emit_conv_pair(b, (0, 1), xes_next)
            xes2 = emit_build_xe_pair(b, (2, 3))
            emit_conv_pair(b, (2, 3), xes2)
            if b + 1 < NB:
                emit_p1_ca(b + 1)
                xes_next = emit_build_xe_pair(b + 1, (0, 1))
            thr = emit_bisect(b)
            emit_mask_spill(b, thr)
            emit_p3(b)

    nc.compile()
    return nc


_NC_CACHE = {}


def kernel(x_in: np.ndarray, threshold_param: np.ndarray) -> np.ndarray:
    from concourse import bass_utils
    q = np.float32(np.asarray(threshold_param).reshape(-1)[0])
    k, g, lw = quantile_host_params(q)
    key = (k, g, lw)
    if key not in _NC_CACHE:
        _NC_CACHE[key] = build_nc(k, g, lw)
    nc = _NC_CACHE[key]
    hc = host_consts()
    cmaps = {n.lower(): hc[n] for n in CONST_NAMES}
    x_in = np.ascontiguousarray(x_in, dtype=np.float32)
    in_maps = [{"x": x_in[NB * c:NB * (c + 1)], **cmaps} for c in range(NCORES)]
    res = bass_utils.run_bass_kernel_spmd(nc, in_maps,
                                          core_ids=list(range(NCORES)))
    return np.concatenate([res.results[c]["out"] for c in range(NCORES)],
                          axis=0)
